# revision 1
# baseline (speedup 1.0000x reference)
"""Trainium2 Bass kernel for a cross-attention layer (CoAttention + RMSNorm output).

Reference computation (per batch b):
    q = hidden @ Wq.T + bq ; k = kv @ Wk.T + bk ; v = kv @ Wv.T + bv
    probs = softmax(q k^T / sqrt(64))
    ctx = probs @ v
    out = RMSNorm(ctx @ Wo.T + bo + hidden) * gamma

Sharding: 8 cores = 4 batches x 2 query-row halves. Each core produces
out[b, half*1024:(half+1)*1024, :] with no cross-core communication
(K/V projections are duplicated within a batch pair).

Per-core pipeline (all matmuls keep contraction dim on SBUF partitions,
enabled by host-side transposes of activations/weights):
  A) QT[o, s]  = WqT.T @ xqT          (fp32r, +bq via per-partition bias)
  B) KT[o, s]  = WkT.T @ xkvT -> DRAM scratch (streamed back per head pair)
  C) V[s, o]   = xkvT.T @ WvT -> SBUF resident as bf16 [kv, head, 64+1]
                 (65th column = 1.0: folds softmax row-sums into ctx matmul)
  D) per head: S^T[kv, sq] = KT_h.T @ QT_h (fp32r; two heads packed in the
     128-row PE array via base-partition 0/64), E = exp(S^T/8) in bf16 on ACT,
     ctx+^T[65, sq] += V+_h.T @ E (bf16); row 64 accumulates sum(exp).
     Normalize: R = broadcast(1/rowsum) via a K=1 PE matmul; ctxT = ctx+ * R.
  E) O[s, o] = ctxT.T @ WoT (bf16) + resid(+bo); RMSNorm * gamma; DMA out.
"""

import numpy as np
import ml_dtypes

import concourse.bass as bass
import concourse.mybir as mybir
from concourse import bass_utils, tile

P = 128
H = 1024
NH = 16
HD = 64
B = 4
SQ = 2048
SQL = 1024  # per-core query rows
SKV = 2048
KC = H // P  # 8 contraction chunks of 128
NKV = SKV // P  # 16 kv chunks
EPS = 1e-6

F32 = mybir.dt.float32
F32R = mybir.dt.float32r
BF16 = mybir.dt.bfloat16
AF = mybir.ActivationFunctionType
OP = mybir.AluOpType

N_CORES = 8


class SplitDrainTileContext(tile.TileContext):
    """TileContext whose tail drain splits sem waits across chained drains.

    The walrus build in this container rejects CTRL instructions that carry
    more than one sync wait; the stock tail drain aggregates the whole global
    clock onto a single Drain instruction.
    """

    MAXW = 1

    def _drain_and_barrier(self, tick_clock, wait_clock):
        drain_inst = self.nc.sync.drain()
        wait_clock.add_sem_waits(
            drain_inst.ins, tile.ScopedClock({None: tick_clock.global_clock})
        )
        si = drain_inst.ins.sync_info
        if si is not None and si.on_wait and len(si.on_wait) > self.MAXW:
            waits = list(si.on_wait)
            drain_inst.ins.sync_info = mybir.SyncInfo(
                on_wait=waits[: self.MAXW], on_update=list(si.on_update or [])
            )
            rest = waits[self.MAXW :]
            for i in range(0, len(rest), self.MAXW):
                d2 = self.nc.sync.drain()
                d2.ins.sync_info = mybir.SyncInfo(
                    on_wait=rest[i : i + self.MAXW], on_update=[]
                )
        self.nc.all_engine_barrier()
        assert self.sems is not None
        popped = self.nc._tile_sem_poison_stack.pop()
        assert popped is self._sem_poison
        self.nc.clear_and_free_semaphores(list(self.sems.allocated().values()))
        self.nc.all_engine_barrier()


def _split_sync_waits(nc, maxw=1):
    """Hoist excess per-instruction sem waits onto preceding same-engine NoOps.

    The walrus build in this container rejects instructions carrying more
    than one sync wait command (any opcode family)."""
    n = 0
    tail_bb = nc.cur_bb.bb
    for f in nc.m.functions:
        for bb in f.blocks:
            il = bb.instructions
            i = 0
            while i < len(il):
                inst = il[i]
                si = inst.sync_info
                if si is not None and si.on_wait and len(si.on_wait) > maxw:
                    waits = list(si.on_wait)
                    keep = waits[-maxw:]
                    extra = waits[:-maxw]
                    inst.sync_info = mybir.SyncInfo(
                        on_wait=keep, on_update=list(si.on_update or [])
                    )
                    for w in extra:
                        b = nc.engines[inst.engine].nop(nofuse=True)
                        carrier = b.ins
                        popped = tail_bb.instructions.pop()
                        assert popped is carrier, "nop landed in unexpected block"
                        carrier.sync_info = mybir.SyncInfo(on_wait=[w], on_update=[])
                        il.insert(i, carrier)
                        i += 1
                        n += 1
                i += 1
    return n


def build_core_kernel(split_waits=True):
    nc = bass.Bass("TRN2", target_bir_lowering=False, debug=False, num_devices=1)

    def inp(name, shape, dt=F32):
        return nc.dram_tensor(name, shape, dt, kind="ExternalInput").ap()

    xqT = inp("xqT", [H, SQL], BF16)
    xkvT = inp("xkvT", [H, SKV], BF16)
    wqT = inp("wqT", [H, H], BF16)
    wkT = inp("wkT", [H, H], BF16)
    wvT = inp("wvT", [H, H], BF16)
    woT = inp("woT", [H, H], BF16)
    bqc = inp("bqc", [P, KC])
    bkc = inp("bkc", [P, KC])
    bvr = inp("bvr", [P, H])
    resid = inp("resid", [SQL, H])
    gam = inp("gam", [P, H])
    onesd = inp("onesd", [1, HD], F32R)
    out = nc.dram_tensor("out", [SQL, H], F32, kind="ExternalOutput").ap()

    with SplitDrainTileContext(nc) as tc:
        with (
            nc.allow_low_precision(reason="bf16 staging of fp32 data"),
            tc.tile_pool(name="pers", bufs=1) as pers,
            tc.tile_pool(name="wt", bufs=2) as wpool,
            tc.tile_pool(name="xs", bufs=2) as xpool,
            tc.tile_pool(name="sm", bufs=8) as smpool,
            tc.tile_pool(name="r4", bufs=3) as rpool,
            tc.tile_pool(name="tiny", bufs=4) as tpool,
            tc.tile_pool(name="psc", bufs=2, space="PSUM") as spool,
            tc.tile_pool(name="pcx", bufs=2, space="PSUM") as cpool,
            tc.tile_pool(name="pmi", bufs=2, space="PSUM") as mpool,
        ):
            # --- persistent tiles -------------------------------------------------
            qt = pers.tile([P, KC, SQL], BF16, name="qt")          # Q^T  [o, s]
            ktall = pers.tile([P, KC, SKV], BF16, name="ktall")    # K^T  [o, s]
            v_sb = pers.tile([P, NKV, NH, HD + 1], BF16, name="v_sb")
            ctxT = pers.tile([P, KC, SQL], BF16, name="ctxT")      # ctx^T [c, s]
            accum_o = pers.tile([P, KC, H], BF16, name="accum_o")  # O partial sums
            bq_sb = pers.tile([P, KC], F32, name="bq_sb")
            bk_sb = pers.tile([P, KC], F32, name="bk_sb")
            bv_sb = pers.tile([P, H], F32, name="bv_sb")
            gam_sb = pers.tile([P, H], F32, name="gam_sb")
            ones1 = pers.tile([1, HD], F32R, name="ones1")
            eps_sb = pers.tile([P, 1], F32, name="eps_sb")
            nc.vector.memset(eps_sb, EPS)

            nc.sync.dma_start(bq_sb, bqc)
            nc.sync.dma_start(bk_sb, bkc)
            nc.sync.dma_start(bv_sb, bvr)
            nc.sync.dma_start(gam_sb, gam)
            nc.sync.dma_start(ones1, onesd)
            nc.vector.memset(v_sb[:, :, :, HD], 1.0)

            def load_w(wT, name):
                w = wpool.tile([P, KC, H], BF16, tag="wt", name=name)
                for ic in range(KC):
                    nc.sync.dma_start(w[:, ic, :], wT[ic * P : (ic + 1) * P, :])
                return w

            # --- phase A: Q^T = WqT.T @ xqT (+bq) ---------------------------------
            wq = load_w(wqT, "wq")
            for sc in range(SQL // 512):
                xq = xpool.tile([P, KC, 512], BF16, tag="xs", name="xq")
                for ic in range(KC):
                    nc.sync.dma_start(
                        xq[:, ic, :], xqT[ic * P : (ic + 1) * P, sc * 512 : (sc + 1) * 512]
                    )
                for oc in range(KC):
                    ps = cpool.tile([P, 512], F32, tag="pcx", name="ps_q")
                    for ic in range(KC):
                        nc.tensor.matmul(
                            ps,
                            wq[:, ic, oc * P : (oc + 1) * P],
                            xq[:, ic, :],
                            start=(ic == 0),
                            stop=(ic == KC - 1),
                        )
                    nc.vector.tensor_scalar_add(
                        qt[:, oc, sc * 512 : (sc + 1) * 512], ps, bq_sb[:, oc : oc + 1]
                    )

            # --- phase B: K^T = WkT.T @ xkvT (+bk), resident ----------------------
            wk = load_w(wkT, "wk")
            for sc in range(SKV // 512):
                xkv = xpool.tile([P, KC, 512], BF16, tag="xs", name="xkv")
                for ic in range(KC):
                    nc.sync.dma_start(
                        xkv[:, ic, :], xkvT[ic * P : (ic + 1) * P, sc * 512 : (sc + 1) * 512]
                    )
                for oc in range(KC):
                    ps = cpool.tile([P, 512], F32, tag="pcx", name="ps_k")
                    for ic in range(KC):
                        nc.tensor.matmul(
                            ps,
                            wk[:, ic, oc * P : (oc + 1) * P],
                            xkv[:, ic, :],
                            start=(ic == 0),
                            stop=(ic == KC - 1),
                        )
                    nc.vector.tensor_scalar_add(
                        ktall[:, oc, sc * 512 : (sc + 1) * 512], ps, bk_sb[:, oc : oc + 1]
                    )

            # --- phase C: V = xkvT.T @ WvT (+bv) -> bf16 SBUF ---------------------
            wv = load_w(wvT, "wv")

            def vproj_chunks(oc2, pool):
                """V-proj half; one closed PSUM group (8 MMs + copyback) per yield."""
                for sc in range(SKV // 512):
                    xkv2 = xpool.tile([P, KC, 512], BF16, tag="xs", name="xkv2")
                    for ic in range(KC):
                        nc.sync.dma_start(
                            xkv2[:, ic, :],
                            xkvT[ic * P : (ic + 1) * P, sc * 512 : (sc + 1) * 512],
                        )
                    for s2 in range(4):
                        kvc = sc * 4 + s2
                        ps = pool.tile([P, 512], F32, tag="pcx" if pool is cpool else "pmi", name="ps_v")
                        for ic in range(KC):
                            nc.tensor.matmul(
                                ps,
                                xkv2[:, ic, s2 * P : (s2 + 1) * P],
                                wv[:, ic, oc2 * 512 : (oc2 + 1) * 512],
                                start=(ic == 0),
                                stop=(ic == KC - 1),
                            )
                        nc.vector.tensor_tensor(
                            v_sb[:, kvc, oc2 * 8 : (oc2 + 1) * 8, 0:HD],
                            ps.rearrange("p (h d) -> p h d", d=HD),
                            bvr_view(bv_sb, oc2),
                            OP.add,
                        )
                        yield

            for _ in vproj_chunks(0, cpool):
                pass
            wo = load_w(woT, "wo")

            def ochunk_steps(cc):
                for s2 in range(SQL // P):
                    for oc2 in range(2):
                        po = mpool.tile([P, 512], F32, tag="pmi", name="ps_oc")
                        nc.tensor.matmul(
                            po,
                            ctxT[:, cc, s2 * P : (s2 + 1) * P],
                            wo[:, cc, oc2 * 512 : (oc2 + 1) * 512],
                            start=True,
                            stop=True,
                        )
                        dst = accum_o[:, s2, oc2 * 512 : (oc2 + 1) * 512]
                        if cc == 0:
                            nc.vector.tensor_copy(dst, po)
                        else:
                            nc.vector.tensor_tensor(dst, dst, po, OP.add)
                        yield

            def chain(*gens):
                for g in gens:
                    yield from g

            def pads(n):
                for _ in range(n):
                    yield

            def interleave(g, k):
                """yield one step of g, then k pad yields, until g is done."""
                for step in g:
                    yield
                    for _ in range(k):
                        yield

            # --- attention pair ---------------------------------------------------
            pending_norms = []

            def emit_pair(hp, filler):
                for sqc in range(2):
                    cps = [
                        cpool.tile([P, 512], F32, tag="pcx", name="ps_ctx")
                        for _ in range(2)
                    ]
                    prev_e = None

                    def emit_ctx(kvc, e):
                        for h in range(2):
                            nc.tensor.matmul(
                                cps[h][0 : HD + 1, :],
                                v_sb[:, kvc, 2 * hp + h, :],
                                e[:, h * 512 : (h + 1) * 512],
                                start=(kvc == 0),
                                stop=(kvc == NKV - 1),
                            )

                    for kvc in range(NKV):
                        if kvc == 1:
                            while pending_norms:
                                pending_norms.pop(0)()
                        sp = spool.tile([P, 1024], F32, tag="psc", name="ps_sc")
                        for h in range(2):
                            nc.tensor.matmul(
                                sp[:, h * 512 : (h + 1) * 512],
                                ktall[h * HD : (h + 1) * HD, hp, kvc * P : (kvc + 1) * P],
                                qt[
                                    h * HD : (h + 1) * HD,
                                    hp,
                                    sqc * 512 : (sqc + 1) * 512,
                                ],
                                start=True,
                                stop=True,
                            )
                        next(filler, None)
                        e = smpool.tile([P, 1024], BF16, tag="sm", name="e_t")
                        nc.scalar.activation(e, sp, AF.Exp, scale=0.125)
                        if prev_e is not None:
                            emit_ctx(kvc - 1, prev_e)
                        prev_e = e
                    emit_ctx(NKV - 1, prev_e)
                    # reciprocals now (DVE only); broadcast+multiply deferred so
                    # the R matmuls never block the next sq-half's scores on PE
                    recs = []
                    for h in range(2):
                        # 1/x as exp(-ln(x)) on ACT: keeps the slow DVE
                        # iterative divide off the boundary critical path
                        lnt = smpool.tile([1, 512], F32, tag="sm", name="lnt")
                        nc.scalar.activation(lnt, cps[h][HD : HD + 1, :], AF.Ln)
                        rec = smpool.tile([1, 512], F32R, tag="sm", name="rec")
                        nc.scalar.activation(rec, lnt, AF.Exp, scale=-1.0)
                        recs.append(rec)

                    def deferred_norm(hp=hp, sqc=sqc, cps=cps, recs=recs):
                        for h in range(2):
                            rp = mpool.tile([P, 512], F32, tag="pmi", name="ps_r")
                            nc.tensor.matmul(
                                rp[0:HD, :], ones1, recs[h], start=True, stop=True
                            )
                            r_sb = smpool.tile([HD, 512], F32, tag="sm", name="r_sb")
                            nc.vector.tensor_copy(r_sb, rp[0:HD, :])
                            dst = ctxT[
                                h * HD : (h + 1) * HD, hp, sqc * 512 : (sqc + 1) * 512
                            ]
                            if h == 0:
                                nc.vector.tensor_tensor(
                                    dst, cps[h][0:HD, :], r_sb, OP.mult
                                )
                            else:
                                stg = smpool.tile([HD, 512], BF16, tag="sm", name="stg")
                                nc.vector.tensor_tensor(
                                    stg, cps[h][0:HD, :], r_sb, OP.mult
                                )
                                nc.sync.dma_start(dst, stg)

                    pending_norms.append(deferred_norm)

            # --- schedule: pairs with background filler ---------------------------
            # positions (1-indexed, 32 per pair); och(cc) must start > 32*(cc+1)
            # och(cc) must start after pair cc's deferred norms (emitted at
            # kvc==1 of the following sq-half, i.e. step 32*(cc+1)+2)
            filler = chain(
                interleave(vproj_chunks(1, mpool), 1),  # 1-32: V1 over pair 0
                pads(6),
                ochunk_steps(0),                        # 39-54
                pads(16),
                ochunk_steps(1),                        # 69-84
                pads(16),
                ochunk_steps(2),                        # 101-116
                pads(16),
                ochunk_steps(3),                        # 133-148
                pads(16),
                ochunk_steps(4),                        # 165-180
                pads(16),
                ochunk_steps(5),                        # 197-212
                pads(16),
                ochunk_steps(6),                        # 229-244
            )
            for hp in range(NH // 2):
                emit_pair(hp, filler)
            while pending_norms:
                pending_norms.pop(0)()
            for _ in filler:
                pass
            for _ in ochunk_steps(NH // 2 - 1):
                pass

            # --- phase E: residual + RMSNorm epilogue -----------------------------
            for s2 in range(SQL // P):
                rs = rpool.tile([P, H], F32, tag="r4", name="rs")
                nc.sync.dma_start(rs, resid[s2 * P : (s2 + 1) * P, :])
                h_sb = rpool.tile([P, H], F32, tag="r4", name="h_sb")
                nc.vector.tensor_tensor(h_sb, accum_o[:, s2, :], rs, OP.add)
                sq = rpool.tile([P, H], F32, tag="r4", name="sq_scratch")
                ss = tpool.tile([P, 1], F32, tag="tiny", name="ss")
                nc.vector.tensor_tensor(sq, h_sb, h_sb, OP.mult)
                nc.vector.tensor_reduce(ss, sq, axis=mybir.AxisListType.X, op=OP.add)
                sr = tpool.tile([P, 1], F32, tag="tiny", name="sr")
                nc.scalar.activation(sr, ss, AF.Sqrt, scale=1.0 / H, bias=eps_sb)
                rr = tpool.tile([P, 1], F32, tag="tiny", name="rr")
                nc.vector.reciprocal(rr, sr)
                nc.vector.tensor_scalar_mul(h_sb, h_sb, rr)
                nc.vector.tensor_tensor(rs, h_sb, gam_sb, OP.mult)
                nc.sync.dma_start(out[s2 * P : (s2 + 1) * P, :], rs)

    if split_waits:
        _split_sync_waits(nc)
    return nc


def bvr_view(bv_sb, oc2):
    return bv_sb[:, oc2 * 512 : (oc2 + 1) * 512].rearrange("p (h d) -> p h d", d=HD)


_NC = None


def _get_nc():
    global _NC
    if _NC is None:
        _NC = build_core_kernel()
    return _NC


def make_in_maps(hidden_states, keyvalue_states, Wq, bq, Wk, bk, Wv, bv, Wo, bo, gamma):
    f = np.float32
    hidden_states = np.asarray(hidden_states, f)
    keyvalue_states = np.asarray(keyvalue_states, f)
    shared = {
        "wqT": np.ascontiguousarray(np.asarray(Wq, f).T).astype(ml_dtypes.bfloat16),
        "wkT": np.ascontiguousarray(np.asarray(Wk, f).T).astype(ml_dtypes.bfloat16),
        "wvT": np.ascontiguousarray(np.asarray(Wv, f).T).astype(ml_dtypes.bfloat16),
        "woT": np.ascontiguousarray(np.asarray(Wo, f).T).astype(ml_dtypes.bfloat16),
        "bqc": np.ascontiguousarray(np.asarray(bq, f).reshape(KC, P).T),
        "bkc": np.ascontiguousarray(np.asarray(bk, f).reshape(KC, P).T),
        "bvr": np.ascontiguousarray(np.tile(np.asarray(bv, f), (P, 1))),
        "gam": np.ascontiguousarray(np.tile(np.asarray(gamma, f), (P, 1))),
        "onesd": np.ones((1, HD), f),
    }
    bo = np.asarray(bo, f)
    in_maps = []
    for core in range(N_CORES):
        b, half = divmod(core, 2)
        hq = hidden_states[b, half * SQL : (half + 1) * SQL, :]
        m = dict(shared)
        m["xqT"] = np.ascontiguousarray(hq.T).astype(ml_dtypes.bfloat16)
        m["xkvT"] = np.ascontiguousarray(keyvalue_states[b].T).astype(ml_dtypes.bfloat16)
        m["resid"] = np.ascontiguousarray(hq + bo)
        in_maps.append(m)
    return in_maps


def _run(in_maps, trace=False, **kwargs):
    nc = _get_nc()
    return bass_utils.run_bass_kernel_spmd(
        nc, in_maps, core_ids=list(range(N_CORES)), trace=trace, **kwargs
    )


def _assemble(res):
    out = np.empty((B, SQ, H), np.float32)
    for core in range(N_CORES):
        b, half = divmod(core, 2)
        out[b, half * SQL : (half + 1) * SQL, :] = res.results[core]["out"]
    return out


def kernel(hidden_states, keyvalue_states, Wq, bq, Wk, bk, Wv, bv, Wo, bo, gamma):
    in_maps = make_in_maps(
        hidden_states, keyvalue_states, Wq, bq, Wk, bk, Wv, bv, Wo, bo, gamma
    )
    return _assemble(_run(in_maps))



# revision 12
# speedup vs baseline: 1.2009x; 1.2009x over previous
"""Trainium2 Bass kernel for a cross-attention layer (CoAttention + RMSNorm output).

Reference computation (per batch b):
    q = hidden @ Wq.T + bq ; k = kv @ Wk.T + bk ; v = kv @ Wv.T + bv
    probs = softmax(q k^T / sqrt(64))
    ctx = probs @ v
    out = RMSNorm(ctx @ Wo.T + bo + hidden) * gamma

Sharding: 8 cores = 4 batches x 2 query-row halves. Each core produces
out[b, half*1024:(half+1)*1024, :] with no cross-core communication
(K/V projections are duplicated within a batch pair).

Per-core pipeline (v2: fp8 DoubleRow matmuls + sqc-outer schedule):
  A) Q/K/V projections in fp8 with perf_mode=DoubleRow (contraction 256 per
     matmul instruction, 2x PE throughput). Q^T/K^T staged bf16 (+bias via
     per-partition scalar); V staged fp8 into v8 with a 65th ones column
     (folds softmax row-sums into the ctx matmul). bv is folded into the
     host-side residual as Wo @ bv (softmax rows sum to 1).
  B) Attention looped sqc-outer (512 query cols), head-pairs inner:
     scores S^T[kv, q] via bf16 row-packed matmul pairs (base partition
     0/64); exp on ACT writes fp8 e8 tiles; ctx accumulated with fp8
     DoubleRow matmuls contracting TWO kv chunks (256) per instruction.
     Row-sum reciprocal via exp(-ln(x)) on ACT; normalization deferred one
     pair (R broadcast matmul + DVE mults).
  C) O projection per 128-row block accumulates all 8 contraction chunks in
     PSUM (bf16 matmuls); epilogue (+resid, RMSNorm, *gamma) streams per
     block, with output DMA issued from the gpsimd queue. Blocks for the
     first query half execute as filler during the second half's attention,
     so only ~1/8 of the output drains after the last matmul.
"""

import numpy as np
import ml_dtypes

import concourse.bass as bass
import concourse.mybir as mybir
from concourse import bass_utils, tile

P = 128
H = 1024
NH = 16
HD = 64
B = 4
SQ = 2048
SQL = 1024  # per-core query rows
SKV = 2048
KC = H // P  # 8 contraction chunks of 128
ICP = KC // 2  # 4 contraction pairs for DoubleRow
NKV = SKV // P  # 16 kv chunks
NKP = NKV // 2  # 8 kv chunk pairs
EPS = 1e-6

F32 = mybir.dt.float32
F32R = mybir.dt.float32r
BF16 = mybir.dt.bfloat16
FP8 = mybir.dt.float8e4
AF = mybir.ActivationFunctionType
OP = mybir.AluOpType
PM = mybir.MatmulPerfMode

N_CORES = 8


class SplitDrainTileContext(tile.TileContext):
    """TileContext whose tail drain splits sem waits across chained drains.

    The walrus build in this container rejects CTRL instructions that carry
    more than one sync wait; the stock tail drain aggregates the whole global
    clock onto a single Drain instruction.
    """

    MAXW = 1

    def _drain_and_barrier(self, tick_clock, wait_clock):
        drain_inst = self.nc.sync.drain()
        wait_clock.add_sem_waits(
            drain_inst.ins, tile.ScopedClock({None: tick_clock.global_clock})
        )
        si = drain_inst.ins.sync_info
        if si is not None and si.on_wait and len(si.on_wait) > self.MAXW:
            waits = list(si.on_wait)
            drain_inst.ins.sync_info = mybir.SyncInfo(
                on_wait=waits[: self.MAXW], on_update=list(si.on_update or [])
            )
            rest = waits[self.MAXW :]
            for i in range(0, len(rest), self.MAXW):
                d2 = self.nc.sync.drain()
                d2.ins.sync_info = mybir.SyncInfo(
                    on_wait=rest[i : i + self.MAXW], on_update=[]
                )
        self.nc.all_engine_barrier()
        assert self.sems is not None
        popped = self.nc._tile_sem_poison_stack.pop()
        assert popped is self._sem_poison
        self.nc.clear_and_free_semaphores(list(self.sems.allocated().values()))
        self.nc.all_engine_barrier()


def _split_sync_waits(nc, maxw=1):
    """Hoist excess per-instruction sem waits onto preceding same-engine NoOps.

    The walrus build in this container rejects instructions carrying more
    than one sync wait command (any opcode family)."""
    n = 0
    tail_bb = nc.cur_bb.bb
    for f in nc.m.functions:
        for bb in f.blocks:
            il = bb.instructions
            i = 0
            while i < len(il):
                inst = il[i]
                si = inst.sync_info
                if si is not None and si.on_wait and len(si.on_wait) > maxw:
                    waits = list(si.on_wait)
                    keep = waits[-maxw:]
                    extra = waits[:-maxw]
                    inst.sync_info = mybir.SyncInfo(
                        on_wait=keep, on_update=list(si.on_update or [])
                    )
                    for w in extra:
                        b = nc.engines[inst.engine].nop(nofuse=True)
                        carrier = b.ins
                        popped = tail_bb.instructions.pop()
                        assert popped is carrier, "nop landed in unexpected block"
                        carrier.sync_info = mybir.SyncInfo(on_wait=[w], on_update=[])
                        il.insert(i, carrier)
                        i += 1
                        n += 1
                i += 1
    return n


def build_core_kernel(split_waits=True):
    nc = bass.Bass("TRN2", target_bir_lowering=False, debug=False, num_devices=1)

    def inp(name, shape, dt=F32):
        return nc.dram_tensor(name, shape, dt, kind="ExternalInput").ap()

    xq8 = inp("xq8", [H, SQL], FP8)
    xkv8 = inp("xkv8", [H, SKV], FP8)
    wq8 = inp("wq8", [H, H], FP8)
    wk8 = inp("wk8", [H, H], FP8)
    wv8 = inp("wv8", [H, H], FP8)
    woT = inp("woT", [H, H], BF16)
    bqc = inp("bqc", [P, KC])
    bkc = inp("bkc", [P, KC])
    resid = inp("resid", [SQL, H])
    gam = inp("gam", [P, H])
    onesd = inp("onesd", [1, HD], F32R)
    out = nc.dram_tensor("out", [SQL, H], F32, kind="ExternalOutput").ap()
    DEBUG = bool(int(__import__("os").environ.get("K_DEBUG", "0")))
    if DEBUG:
        dbg_kt = nc.dram_tensor("dbg_kt", [P, KC, SKV], BF16, kind="ExternalOutput").ap()
        dbg_qt = nc.dram_tensor("dbg_qt", [P, KC, SQL], BF16, kind="ExternalOutput").ap()
        dbg_ctxT = nc.dram_tensor("dbg_ctxT", [P, KC, SQL], BF16, kind="ExternalOutput").ap()
        dbg_v8 = nc.dram_tensor("dbg_v8", [P, NKP, 2, NH, 80], FP8, kind="ExternalOutput").ap()

    with SplitDrainTileContext(nc) as tc:
        with (
            nc.allow_low_precision(reason="fp8/bf16 staging of fp32 data"),
            tc.tile_pool(name="pers", bufs=1) as pers,
            tc.tile_pool(name="sm", bufs=6) as smpool,
            tc.tile_pool(name="e8p", bufs=4) as e8pool,
            tc.tile_pool(name="r4", bufs=6) as rpool,
            tc.tile_pool(name="tiny", bufs=4) as tpool,
            tc.tile_pool(name="psc", bufs=2, space="PSUM") as spool,
            tc.tile_pool(name="pcx", bufs=2, space="PSUM") as cpool,
            tc.tile_pool(name="pmi", bufs=2, space="PSUM") as mpool,
        ):
            # --- persistent tiles -------------------------------------------------
            qt = pers.tile([P, KC, SQL], BF16, name="qt")          # Q^T  [o, s]
            kt = pers.tile([P, KC, SKV], BF16, name="kt")          # K^T  [o, s]
            v8 = pers.tile([P, NKP, 2, NH, 80], FP8, name="v8")    # V + ones col
            ctxT = pers.tile([P, KC, SQL], BF16, name="ctxT")      # ctx^T [c, s]
            xq8_sb = pers.tile([P, ICP, 2, SQL], FP8, name="xq8_sb")
            xkv8_sb = pers.tile([P, ICP, 2, SKV], FP8, name="xkv8_sb")
            wq8_sb = pers.tile([P, ICP, 2, H], FP8, name="wq8_sb")
            wk8_sb = pers.tile([P, ICP, 2, H], FP8, name="wk8_sb")
            wv8_sb = pers.tile([P, ICP, 2, H], FP8, name="wv8_sb")
            wo_sb = pers.tile([P, KC, H], BF16, name="wo_sb")
            bq_sb = pers.tile([P, KC], F32, name="bq_sb")
            bk_sb = pers.tile([P, KC], F32, name="bk_sb")
            gam_sb = pers.tile([P, H], F32, name="gam_sb")
            ones1 = pers.tile([1, HD], F32R, name="ones1")
            eps_sb = pers.tile([P, 1], F32, name="eps_sb")
            nc.vector.memset(eps_sb, EPS)
            nc.vector.memset(v8[:, :, :, :, HD], 1.0)

            nc.sync.dma_start(bq_sb, bqc)
            nc.sync.dma_start(bk_sb, bkc)
            nc.sync.dma_start(gam_sb, gam)
            nc.sync.dma_start(ones1, onesd)

            def load8(dst, src, eng):
                for icp in range(ICP):
                    for t2 in range(2):
                        r = (2 * icp + t2) * P
                        eng.dma_start(dst[:, icp, t2, :], src[r : r + P, :])

            # spread the 9MB of input loads over four DGE queues so the
            # prelude projections aren't serialized behind one queue
            load8(xkv8_sb, xkv8, nc.sync)
            load8(wk8_sb, wk8, nc.scalar)
            load8(wv8_sb, wv8, nc.gpsimd)
            load8(xq8_sb, xq8, nc.scalar)
            load8(wq8_sb, wq8, nc.sync)
            for ic in range(KC):
                nc.gpsimd.dma_start(wo_sb[:, ic, :], woT[ic * P : (ic + 1) * P, :])

            # --- projection work units (fp8 DoubleRow, 4 MMs per 512-col tile) ----
            def kproj_units(sc):
                for oc in range(KC):
                    ps = spool.tile([P, 1024], F32, tag="psc", name="ps_k")
                    pv = ps[:, 0:512]
                    for icp in range(ICP):
                        nc.tensor.matmul(
                            pv,
                            wk8_sb[:, icp, :, oc * P : (oc + 1) * P],
                            xkv8_sb[:, icp, :, sc * 512 : (sc + 1) * 512],
                            start=(icp == 0),
                            stop=(icp == ICP - 1),
                            perf_mode=PM.DoubleRow,
                        )
                    nc.vector.tensor_scalar_add(
                        kt[:, oc, sc * 512 : (sc + 1) * 512], pv, bk_sb[:, oc : oc + 1]
                    )
                    yield

            def qproj_units(sqc, ocs):
                for oc in ocs:
                    ps = spool.tile([P, 1024], F32, tag="psc", name="ps_q")
                    pv = ps[:, 0:512]
                    for icp in range(ICP):
                        nc.tensor.matmul(
                            pv,
                            wq8_sb[:, icp, :, oc * P : (oc + 1) * P],
                            xq8_sb[:, icp, :, sqc * 512 : (sqc + 1) * 512],
                            start=(icp == 0),
                            stop=(icp == ICP - 1),
                            perf_mode=PM.DoubleRow,
                        )
                    nc.vector.tensor_scalar_add(
                        qt[:, oc, sqc * 512 : (sqc + 1) * 512], pv, bq_sb[:, oc : oc + 1]
                    )
                    yield

            def vproj_units(sc, oc2):
                for s2 in range(4):
                    kvc = sc * 4 + s2
                    ps = spool.tile([P, 1024], F32, tag="psc", name="ps_v")
                    pv = ps[:, 0:512]
                    for icp in range(ICP):
                        nc.tensor.matmul(
                            pv,
                            xkv8_sb[:, icp, :, kvc * P : (kvc + 1) * P],
                            wv8_sb[:, icp, :, oc2 * 512 : (oc2 + 1) * 512],
                            start=(icp == 0),
                            stop=(icp == ICP - 1),
                            perf_mode=PM.DoubleRow,
                        )
                    nc.vector.tensor_copy(
                        v8[:, kvc // 2, kvc % 2, oc2 * 8 : (oc2 + 1) * 8, 0:HD],
                        pv.rearrange("p (h d) -> p h d", d=HD),
                    )
                    yield

            # --- O projection + RMSNorm epilogue per 128-row block ---------------
            def oproj_units(sqc):
                for s2 in range(4):
                    base = sqc * 512 + s2 * P
                    rs = rpool.tile([P, H], F32, tag="r4", name="rs")
                    nc.sync.dma_start(rs, resid[base : base + P, :])
                    pos = []
                    for oc2 in range(2):
                        po = mpool.tile([P, 512], F32, tag="pmi", name="po")
                        for cc in range(KC):
                            nc.tensor.matmul(
                                po,
                                ctxT[:, cc, base : base + P],
                                wo_sb[:, cc, oc2 * 512 : (oc2 + 1) * 512],
                                start=(cc == 0),
                                stop=(cc == KC - 1),
                            )
                        pos.append(po)
                        yield
                    hs = rpool.tile([P, H], F32, tag="r4", name="hs")
                    for oc2 in range(2):
                        nc.vector.tensor_tensor(
                            hs[:, oc2 * 512 : (oc2 + 1) * 512],
                            pos[oc2],
                            rs[:, oc2 * 512 : (oc2 + 1) * 512],
                            OP.add,
                        )
                    sq = rpool.tile([P, H], F32, tag="r4", name="sq_scratch")
                    ss = tpool.tile([P, 1], F32, tag="tiny", name="ss")
                    nc.vector.tensor_tensor(sq, hs, hs, OP.mult)
                    nc.vector.tensor_reduce(ss, sq, axis=mybir.AxisListType.X, op=OP.add)
                    sr = tpool.tile([P, 1], F32, tag="tiny", name="sr")
                    nc.scalar.activation(sr, ss, AF.Sqrt, scale=1.0 / H, bias=eps_sb)
                    rr = tpool.tile([P, 1], F32, tag="tiny", name="rr")
                    nc.vector.reciprocal(rr, sr)
                    nc.vector.tensor_scalar_mul(hs, hs, rr)
                    ot = rpool.tile([P, H], F32, tag="r4", name="ot")
                    nc.vector.tensor_tensor(ot, hs, gam_sb, OP.mult)
                    nc.gpsimd.dma_start(out[base : base + P, :], ot)
                    yield

            def chain(*gens):
                for g in gens:
                    yield from g

            def pads(n):
                for _ in range(n):
                    yield

            # --- attention pair (2 heads, 512 query cols) -------------------------
            pending_norms = []

            def emit_pair(hp, sqc, filler):
                cps = [
                    cpool.tile([P, 512], F32, tag="pcx", name="ps_ctx")
                    for _ in range(2)
                ]
                def emit_ctx(kp, e8t):
                    for h in range(2):
                        nc.tensor.matmul(
                            cps[h][0 : HD + 1, :],
                            v8[:, kp, :, 2 * hp + h, 0 : HD + 1],
                            e8t[:, :, h, :],
                            start=(kp == 0),
                            stop=(kp == NKP - 1),
                            perf_mode=PM.DoubleRow,
                        )

                prev_e = None
                for kp in range(NKP):
                    # the previous pair's deferred norm reads its cps PSUM;
                    # it must be emitted before this pair's first ctx matmul
                    # (start=True) recycles those PSUM slots
                    if kp == 1:
                        while pending_norms:
                            pending_norms.pop(0)()
                    e8t = e8pool.tile([P, 2, 2, 512], FP8, tag="e8", name="e8t")
                    for t in range(2):
                        kvc = 2 * kp + t
                        sp = spool.tile([P, 1024], F32, tag="psc", name="ps_sc")
                        for h in range(2):
                            nc.tensor.matmul(
                                sp[:, h * 512 : (h + 1) * 512],
                                kt[h * HD : (h + 1) * HD, hp, kvc * P : (kvc + 1) * P],
                                qt[
                                    h * HD : (h + 1) * HD,
                                    hp,
                                    sqc * 512 : (sqc + 1) * 512,
                                ],
                                start=True,
                                stop=True,
                            )
                        next(filler, None)
                        nc.scalar.activation(
                            e8t[:, t, :, :].rearrange("p a b -> p (a b)"),
                            sp,
                            AF.Exp,
                            scale=0.125,
                        )
                    if prev_e is not None:
                        emit_ctx(kp - 1, prev_e)
                    prev_e = e8t
                    next(filler, None)
                emit_ctx(NKP - 1, prev_e)
                # row-sum reciprocals now on ACT: 1/x = exp(-ln(x)); broadcast
                # + multiply deferred one pair so the R matmuls never block the
                # next pair's scores on PE
                recs = []
                for h in range(2):
                    lnt = smpool.tile([1, 512], F32, tag="sm", name="lnt")
                    nc.scalar.activation(lnt, cps[h][HD : HD + 1, :], AF.Ln)
                    rec = smpool.tile([1, 512], F32R, tag="rec", bufs=4, name="rec")
                    nc.scalar.activation(rec, lnt, AF.Exp, scale=-1.0)
                    recs.append(rec)

                def deferred_norm(hp=hp, sqc=sqc, cps=cps, recs=recs):
                    for h in range(2):
                        rp = mpool.tile([P, 512], F32, tag="pmi", name="ps_r")
                        nc.tensor.matmul(
                            rp[0:HD, :], ones1, recs[h], start=True, stop=True
                        )
                        r_sb = smpool.tile([HD, 512], F32, tag="sm", name="r_sb")
                        nc.vector.tensor_copy(r_sb, rp[0:HD, :])
                        dst = ctxT[
                            h * HD : (h + 1) * HD, hp, sqc * 512 : (sqc + 1) * 512
                        ]
                        if h == 0:
                            nc.vector.tensor_tensor(
                                dst, cps[h][0:HD, :], r_sb, OP.mult
                            )
                        else:
                            stg = smpool.tile([HD, 512], BF16, tag="sm", name="stg")
                            nc.vector.tensor_tensor(
                                stg, cps[h][0:HD, :], r_sb, OP.mult
                            )
                            nc.sync.dma_start(dst, stg)

                pending_norms.append(deferred_norm)

            # --- schedule ---------------------------------------------------------
            import os as _os

            SCHED = _os.environ.get("K_SCHED", "A")
            if SCHED == "A":
                # safe: all projections ahead of attention
                for _ in chain(
                    kproj_units(0),
                    kproj_units(1),
                    kproj_units(2),
                    kproj_units(3),
                    vproj_units(0, 0),
                    vproj_units(1, 0),
                    vproj_units(2, 0),
                    vproj_units(3, 0),
                    vproj_units(0, 1),
                    vproj_units(1, 1),
                    vproj_units(2, 1),
                    vproj_units(3, 1),
                    qproj_units(0, list(range(KC))),
                    qproj_units(1, list(range(KC))),
                ):
                    pass
                filler0 = pads(0)
            else:
                # racing: V fully preluded (its DoubleRow-weights read spans
                # both k-tile slots, the pattern implicated in the sqc0 NaN);
                # K (beyond the first block) and Q race sqc0's attention
                for _ in chain(
                    vproj_units(0, 0),
                    vproj_units(1, 0),
                    vproj_units(2, 0),
                    vproj_units(3, 0),
                    vproj_units(0, 1),
                    vproj_units(1, 1),
                    vproj_units(2, 1),
                    vproj_units(3, 1),
                    kproj_units(0),
                    qproj_units(0, [0]),
                ):
                    pass
                filler0 = chain(
                    kproj_units(1),
                    qproj_units(0, [1]),
                    kproj_units(2),
                    qproj_units(0, [2, 3]),
                    kproj_units(3),
                    qproj_units(0, [4, 5, 6, 7]),
                    qproj_units(1, list(range(KC))),
                )
            for hp in range(NH // 2):
                emit_pair(hp, 0, filler0)
            for _ in filler0:
                pass

            def interleave(g, k):
                for _ in g:
                    yield
                    for _ in range(k):
                        yield

            # sqc 1: O projection + epilogue for sqc 0 rides as filler
            filler1 = chain(
                pads(16),
                interleave(oproj_units(0), 12),
            )
            for hp in range(NH // 2):
                emit_pair(hp, 1, filler1)
            while pending_norms:
                pending_norms.pop(0)()
            for _ in filler1:
                pass

            # tail: O projection + epilogue for sqc 1
            for _ in oproj_units(1):
                pass

            if DEBUG:
                nc.sync.dma_start(dbg_kt, kt)
                nc.sync.dma_start(dbg_qt, qt)
                nc.sync.dma_start(dbg_ctxT, ctxT)
                nc.sync.dma_start(dbg_v8, v8)

    if split_waits:
        _split_sync_waits(nc)
    return nc


_NC = None


def _get_nc():
    global _NC
    if _NC is None:
        _NC = build_core_kernel()
    return _NC


def make_in_maps(hidden_states, keyvalue_states, Wq, bq, Wk, bk, Wv, bv, Wo, bo, gamma):
    f = np.float32
    f8 = ml_dtypes.float8_e4m3
    hidden_states = np.asarray(hidden_states, f)
    keyvalue_states = np.asarray(keyvalue_states, f)
    Wof = np.asarray(Wo, f)
    shared = {
        "wq8": np.ascontiguousarray(np.asarray(Wq, f).T).astype(f8),
        "wk8": np.ascontiguousarray(np.asarray(Wk, f).T).astype(f8),
        "wv8": np.ascontiguousarray(np.asarray(Wv, f).T).astype(f8),
        "woT": np.ascontiguousarray(Wof.T).astype(ml_dtypes.bfloat16),
        "bqc": np.ascontiguousarray(np.asarray(bq, f).reshape(KC, P).T),
        "bkc": np.ascontiguousarray(np.asarray(bk, f).reshape(KC, P).T),
        "gam": np.ascontiguousarray(np.tile(np.asarray(gamma, f), (P, 1))),
        "onesd": np.ones((1, HD), f),
    }
    # bv folded through the O projection (softmax rows sum to 1): +Wo @ bv
    resid_vec = np.asarray(bo, f) + Wof @ np.asarray(bv, f)
    in_maps = []
    for core in range(N_CORES):
        b, half = divmod(core, 2)
        hq = hidden_states[b, half * SQL : (half + 1) * SQL, :]
        m = dict(shared)
        m["xq8"] = np.ascontiguousarray(hq.T).astype(f8)
        m["xkv8"] = np.ascontiguousarray(keyvalue_states[b].T).astype(f8)
        m["resid"] = np.ascontiguousarray(hq + resid_vec)
        in_maps.append(m)
    return in_maps


def _run(in_maps, trace=False, **kwargs):
    nc = _get_nc()
    return bass_utils.run_bass_kernel_spmd(
        nc, in_maps, core_ids=list(range(N_CORES)), trace=trace, **kwargs
    )


def _assemble(res):
    out = np.empty((B, SQ, H), np.float32)
    for core in range(N_CORES):
        b, half = divmod(core, 2)
        out[b, half * SQL : (half + 1) * SQL, :] = res.results[core]["out"]
    return out


def kernel(hidden_states, keyvalue_states, Wq, bq, Wk, bk, Wv, bv, Wo, bo, gamma):
    in_maps = make_in_maps(
        hidden_states, keyvalue_states, Wq, bq, Wk, bk, Wv, bv, Wo, bo, gamma
    )
    return _assemble(_run(in_maps))
